# revision 20
# baseline (speedup 1.0000x reference)
"""Trainium2 Bass kernel for BERT-style CLS attention head (v2: fp16 dual-layout).

Model (see harness reference):
  q/k/v projections of hidden [B=16, S=1024, H=768], 8 heads x 96,
  softmax attention, but ONLY the CLS token (query position 0) feeds the
  output projection  out = relu(ctx[:, 0] @ Wo + bo)  with Wo [768, 4].

Algebraic structure exploited on-device (per batch b):
  q~      = X[0] @ Wq + bq                  (only row 0 of Q needed)
  Qblk    [768, 16] = diag-blocked q~/sqrt(96)
  Z^T     [16, 768] = Qblk.T @ Wk^T         (K-projection collapses to a
                                             rank-16 op; bk cancels in
                                             softmax)
  scores  [8, 1024] = Z.T @ X^T             (X^T comes pre-transposed
                                             from DRAM - no on-chip
                                             transposes of X at all)
  probs   = exp(scores - 4)                 (unnormalized; shift cancels)
  pt      = probs^T * mask                  (mask folded into the PSUM
                                             evacuation multiply)
  r       [8, 769]  = pt.T @ [X | 1]       (ones-column of X makes
                                             r[:,768] = rowsum -> softmax
                                             denominator for free)
  out     [4]       = relu(sum_hc rt*g/rho + boeff)  (DVE mult-reduce +
                                             one fp32 matmul; G_h =
                                             Wv_h @ Wo_h host-fused)

All streamed tensors are fp16 (host-side cast + layout only; fp32
accumulation in PSUM). Per-core HBM traffic ~8.6 MB -> ~24 us DMA bound.
Sharding: data-parallel over batch, 2 batches per core on 8 cores.
"""

import numpy as np

from concourse import bacc
import concourse.mybir as mybir
import concourse.tile as tile
from concourse.bass import _add_dep_helper
from concourse.bass_utils import run_bass_kernel_spmd

F32 = mybir.dt.float32
import ml_dtypes
NP16 = ml_dtypes.bfloat16
F16 = mybir.dt.bfloat16
F32R = mybir.dt.float32r


def _r(ap):
    return ap.bitcast(F32R)

B, S, H = 16, 1024, 768
NH, DH, O = 8, 96, 4
NCORES = 8
BL = B // NCORES          # 2 batches per core
C6 = H // 128             # 6 hidden chunks of 128
K8 = S // 128             # 8 sequence chunks of 128
HP = 772                  # padded hidden: col 768 = 1.0 (rowsum), 769.. = 0
RCOL = H                  # index of the ones column in padded X

# kw16 packing [128, .] fp16: ident | x0t | g48
KW_IDENT = 0
KW_X0T = 128                       # x0t[p, c*BL + b]
KW_G = KW_X0T + C6 * BL            # 140; g48[p, o*48 + c*NH + h]
KW_ONE = KW_G + O * C6 * NH        # 332: fp16 ones column
KW_LEN = KW_ONE + 4                # 336

# kw32 packing [128, .] fp32: qmask | ones col | boeff (partition 0)
KV_QMASK = 0                       # qmask[p, c*NH + h]
KV_ONES = C6 * NH                  # 48
KV_BOEFF = KV_ONES + 1             # 49 (partition 0 only)
KV_NEG4 = KV_BOEFF + O             # 53: exp bias (-4.0, all partitions)
KV_ID32 = KV_NEG4 + 1              # 54
KV_LEN = KV_ID32 + 128             # 182


def build_program():
    nc = bacc.Bacc(None)

    x_d = nc.declare_dram_parameter("x", [BL, 128, K8, HP], F16, isOutput=False)
    xt_d = nc.declare_dram_parameter("xt", [BL, 128, C6, S], F16, isOutput=False)
    wq_d = nc.declare_dram_parameter("wq", [128, C6, H], F16, isOutput=False)
    wkt_d = nc.declare_dram_parameter("wkt", [128, C6, H], F16, isOutput=False)
    kw16_d = nc.declare_dram_parameter("kw16", [128, KW_LEN], F16, isOutput=False)
    kw32_d = nc.declare_dram_parameter("kw32", [128, KV_LEN], F32, isOutput=False)
    am_d = nc.declare_dram_parameter("am", [128, BL * K8], F16, isOutput=False)
    bq_d = nc.declare_dram_parameter("bq2", [BL, H], F32, isOutput=False)
    out_d = nc.declare_dram_parameter("out", [BL, O], F32, isOutput=True)

    with tile.TileContext(nc) as tc:
        with (
            tc.tile_pool(name="konst", bufs=1) as kp,
            tc.tile_pool(name="work", bufs=1) as wp,
            tc.tile_pool(name="tps", bufs=2, space="PSUM") as tpsp,
            tc.tile_pool(name="acc", bufs=2, space="PSUM") as accp,
            tc.tile_pool(name="jnk", bufs=1, space="PSUM") as jp,
            tc.tile_pool(name="oup", bufs=1, space="PSUM") as op_,
        ):
            # ---- persistent SBUF tiles ----
            kw16 = kp.tile([128, KW_LEN], F16)
            kw32 = kp.tile([128, KV_LEN], F32)
            am16 = kp.tile([128, BL * K8], F16)
            bq32 = kp.tile([BL, H], F32)
            wq_sb = kp.tile([128, C6, H], F16)
            wkt_sb = kp.tile([128, C6, H], F16)
            x_sb = kp.tile([128, BL, K8, HP], F16)
            xt_sb = kp.tile([128, BL, C6, S], F16)

            ident = kw16[:, KW_IDENT : KW_IDENT + 128]
            x0t_v = kw16[:, KW_X0T : KW_G].rearrange("p (c b) -> p c b", c=C6)
            g48_v = kw16[:, KW_G : KW_ONE].rearrange("p (o a) -> p o a", o=O)
            qmask_v = kw32[:, KV_QMASK : KV_ONES].rearrange("p (c h) -> p c h", c=C6)
            ones_v = kw32[:, KV_ONES : KV_ONES + 1]
            one16_v = kw16[:, KW_ONE : KW_ONE + 1]
            boeff_v = kw32[0:1, KV_BOEFF : KV_BOEFF + O]
            id32 = kw32[:, KV_ID32 : KV_ID32 + 128]
            neg4_v = kw32[:, KV_NEG4 : KV_NEG4 + 1]

            # ---- DMA queue (HWDGE; priority-chained, 2 in flight) ----
            dmas = []
            dmas.append(nc.sync.dma_start(out=kw16[:, :], in_=kw16_d[:, :]))
            dmas.append(nc.sync.dma_start(out=kw32[:, :], in_=kw32_d[:, :]))
            dmas.append(nc.sync.dma_start(out=am16[:, :], in_=am_d[:, :]))
            dmas.append(nc.sync.dma_start(out=bq32[:, :], in_=bq_d[:, :]))
            d_wq = []
            d_wkt = []
            for half, (c0, cn) in enumerate(((0, 3), (3, 6))):
                d_wq.append(
                    nc.sync.dma_start(
                        out=wq_sb[:, c0:cn, :],
                        in_=wq_d[:, c0:cn, :],
                    )
                )
            for half, (c0, cn) in enumerate(((0, 3), (3, 6))):
                d_wkt.append(
                    nc.sync.dma_start(
                        out=wkt_sb[:, c0:cn, :],
                        in_=wkt_d[:, c0:cn, :],
                    )
                )
            dmas.extend(d_wq)
            dmas.extend(d_wkt)

            d_xt = {}   # (b, half) -> dma
            d_x = {}    # (b, piece) -> dma; pieces in s-chunks of 128

            def load_xt(b, nh2):
                d_xt[(b, nh2)] = nc.sync.dma_start(
                    out=xt_sb[:, b, :, 512 * nh2 : 512 * (nh2 + 1)],
                    in_=xt_d[b].rearrange("(c p) s -> p c s", p=128)[
                        :, :, 512 * nh2 : 512 * (nh2 + 1)
                    ],
                )

            def load_x(b, k0, kn):
                d_x[(b, k0)] = nc.sync.dma_start(
                    out=x_sb[:, b, k0:kn, :],
                    in_=x_d[b, :, k0:kn, :],
                )

            def load_xt_full(b):
                d_xt[b] = nc.sync.dma_start(
                    out=xt_sb[:, b, :, :],
                    in_=xt_d[b, :, :, :],
                )

            load_xt_full(0)
            load_xt_full(1)
            load_x(0, 0, 4)
            load_x(0, 4, 8)
            load_x(1, 0, 4)
            load_x(1, 4, 6)
            load_x(1, 6, 8)
            dmas.extend(
                [
                    d_xt[0], d_xt[1], d_x[(0, 0)], d_x[(0, 4)],
                    d_x[(1, 0)], d_x[(1, 4)], d_x[(1, 6)],
                ]
            )
            # keep four transfers in flight: hides the ~2.2us per-transfer
            # HWDGE gen + completion-receipt latency while the single ring's
            # FIFO drain keeps bytes landing in priority order
            for i in range(4, len(dmas)):
                _add_dep_helper(
                    dmas[i].ins, dmas[i - 4].ins, sync=True, reason="dma order"
                )

            # ---- PE warmup (HAM unthrottle) while weights stream ----
            warm_ps = jp.tile([128, KW_LEN], F32)
            for _ in range(12):
                nc.tensor.matmul(warm_ps[:, :], ident, kw16[:, :])

            # ---- q~ = X[:,0,:] @ Wq + bq : [BL, H] ----
            q_ps = accp.tile([BL, H], F32, tag="acc", name="q_ps")
            for c in range(C6):
                for n0, nw in ((0, 512), (512, 256)):
                    nc.tensor.matmul(
                        q_ps[:, n0 : n0 + nw],
                        x0t_v[:, c, :],
                        wq_sb[:, c, n0 : n0 + nw],
                        start=(c == 0),
                        stop=(c == C6 - 1),
                    )
            q_sb = wp.tile([BL, H], F32)
            nc.vector.tensor_add(q_sb[:, :], q_ps[:, :], bq32[:, :])

            # ---- qT via PE transposes, fused into Qblk = qT * headmask ----
            qt_ps = tpsp.tile([128, 512], F32, tag="tps", name="qt_ps")
            for c in range(C6):
                nc.tensor.transpose(
                    qt_ps[:, BL * c : BL * (c + 1)],
                    q_sb[:, 128 * c : 128 * (c + 1)],
                    id32[:BL, :BL],
                )
            qblk = wp.tile([128, C6, BL, NH], F16)
            nc.vector.tensor_mul(
                qblk[:, :, :, :],
                qt_ps[:, : C6 * BL]
                .rearrange("p (c b) -> p c b", c=C6)
                .unsqueeze(3)
                .to_broadcast([128, C6, BL, NH]),
                qmask_v.unsqueeze(2).to_broadcast([128, C6, BL, NH]),
            )

            # ---- Z^T [16, 768] = Qblk.T @ WkT, then Z [768, 16] ----
            zt_ps = accp.tile([BL * NH, H], F32, tag="acc", name="zt_ps")
            for jc in range(C6):
                for n0, nw in ((0, 512), (512, 256)):
                    nc.tensor.matmul(
                        zt_ps[:, n0 : n0 + nw],
                        qblk[:, jc, :, :].rearrange("p b h -> p (b h)"),
                        wkt_sb[:, jc, n0 : n0 + nw],
                        start=(jc == 0),
                        stop=(jc == C6 - 1),
                    )
            zt_sb = wp.tile([BL * NH, H], F32)
            nc.vector.tensor_copy(zt_sb[:, :], zt_ps[:, :])
            z_tps = tpsp.tile([128, 512], F32, tag="tps", name="z_tps")
            for c in range(C6):
                nc.tensor.transpose(
                    z_tps[:, 16 * c : 16 * (c + 1)],
                    zt_sb[:, 128 * c : 128 * (c + 1)],
                    id32[: BL * NH, : BL * NH],
                )
            z_sb = wp.tile([128, C6, BL * NH], F16)
            nc.vector.tensor_copy(z_sb[:, :, :], z_tps[:, : C6 * BL * NH].rearrange("p (c a) -> p c a", c=C6))

            # ---- per-batch attention pipeline ----
            probs = wp.tile([NH, BL, S], F32)
            pt_sb = wp.tile([128, BL, K8, NH], F16)

            def scores_half(b, sc_ps, nh2):
                for ic in range(C6):
                    nc.tensor.matmul(
                        sc_ps[:, 512 * nh2 : 512 * (nh2 + 1)],
                        z_sb[:, ic, NH * b : NH * (b + 1)],
                        xt_sb[:, b, ic, 512 * nh2 : 512 * (nh2 + 1)],
                        start=(ic == 0),
                        stop=(ic == C6 - 1),
                    )

            def exp_half(b, sc_ps, nh2):
                # shift-invariant exp; -4 guards fp16 range (|score| <~ 7)
                nc.scalar.activation(
                    probs[:, b, 512 * nh2 : 512 * (nh2 + 1)],
                    sc_ps[:, 512 * nh2 : 512 * (nh2 + 1)],
                    mybir.ActivationFunctionType.Exp,
                    bias=neg4_v[:NH, :],
                    scale=1.0,
                )

            def pt_block(b):
                pt_ps = tpsp.tile([128, 512], F32, tag="tps", name=f"pt_ps{b}")
                for k in range(K8):
                    nc.tensor.transpose(
                        pt_ps[:, NH * k : NH * (k + 1)],
                        probs[:, b, 128 * k : 128 * (k + 1)],
                        id32[:NH, :NH],
                    )
                # attention mask folded into the PSUM evacuation (exact:
                # exp(score - 10000) == 0 == exp(score) * mask in fp32)
                nc.vector.tensor_mul(
                    pt_sb[:, b, :, :],
                    pt_ps[:, : K8 * NH].rearrange("p (k h) -> p k h", k=K8),
                    am16[:, b * K8 : (b + 1) * K8]
                    .unsqueeze(2)
                    .to_broadcast([128, K8, NH]),
                )

            def rho_block(b):
                rho_ps = op_.tile([1, NH], F32, tag="out", name=f"rho{b}")
                for k in range(K8):
                    nc.tensor.matmul(
                        rho_ps[:, :],
                        one16_v,
                        pt_sb[:, b, k, :],
                        start=(k == 0),
                        stop=(k == K8 - 1),
                    )
                rho_sb = wp.tile([1, NH], F32, name=f"rho_sb{b}")
                nc.vector.tensor_copy(rho_sb[:, :], rho_ps[:, :])
                rt_ps_t = tpsp.tile([128, 512], F32, tag="tps", name=f"rhot{b}")
                nc.tensor.transpose(
                    rt_ps_t[:NH, :1], rho_sb[:, :], id32[:1, :1]
                )
                recip = wp.tile([NH, 1], F32, name=f"recip{b}")
                nc.vector.reciprocal(recip[:, :], rt_ps_t[:NH, :1])
                return recip

            def r_chunks(b, r_ps, k0, kn):
                for k in range(k0, kn):
                    for n0, nw in ((0, 512), (512, HP - 512)):
                        nc.tensor.matmul(
                            r_ps[:, n0 : n0 + nw],
                            pt_sb[:, b, k, :],
                            x_sb[:, b, k, n0 : n0 + nw],
                            start=(k == 0),
                            stop=(k == K8 - 1),
                        )

            def finish(b, r_ps, recip):
                rsc = wp.tile([NH, H], F32, name=f"rsc{b}")
                rt_ps = tpsp.tile([128, 512], F32, tag="tps", name=f"rt_ps{b}")
                for c in range(C6):
                    eng = nc.vector if c % 2 == 0 else nc.scalar
                    if c % 2 == 0:
                        nc.vector.tensor_scalar_mul(
                            rsc[:, 128 * c : 128 * (c + 1)],
                            r_ps[:, 128 * c : 128 * (c + 1)],
                            recip[:, :],
                        )
                    else:
                        nc.scalar.activation(
                            rsc[:, 128 * c : 128 * (c + 1)],
                            r_ps[:, 128 * c : 128 * (c + 1)],
                            mybir.ActivationFunctionType.Copy,
                            bias=0.0,
                            scale=recip[:, :],
                        )
                    nc.tensor.transpose(
                        rt_ps[:, NH * c : NH * (c + 1)],
                        rsc[:, 128 * c : 128 * (c + 1)],
                        id32[:NH, :NH],
                    )
                rt_sb = wp.tile([128, C6 * NH], F32, name=f"rt_sb{b}")
                nc.vector.tensor_copy(rt_sb[:, :], rt_ps[:, : C6 * NH])
                scrap = wp.tile([128, O, C6 * NH], F32, name=f"scrap{b}")
                partials = wp.tile([128, O], F32, name=f"partials{b}")
                nc.vector.tensor_mul(
                    scrap[:, :, :],
                    rt_sb[:, :].unsqueeze(1).to_broadcast([128, O, C6 * NH]),
                    g48_v[:, :, :],
                )
                nc.vector.tensor_reduce(
                    partials[:, :].unsqueeze(2),
                    scrap[:, :, :],
                    mybir.AxisListType.X,
                    mybir.AluOpType.add,
                )
                osum_ps = op_.tile([1, O], F32, tag="out", name=f"osum{b}")
                nc.tensor.matmul(
                    osum_ps[:, :], ones_v, partials[:, :], start=True, stop=True
                )
                out1 = wp.tile([1, O], F32, name=f"out1{b}")
                nc.vector.tensor_add(out1[:, :], osum_ps[:, :], boeff_v)
                out_sb = wp.tile([1, O], F32, name=f"out_sb{b}")
                nc.vector.tensor_scalar_max(out_sb[:, :], out1[:, :], 0.0)
                nc.scalar.dma_start(out=out_d[b : b + 1, :], in_=out_sb[:, :])

            # interleaved two-batch pipeline: PE chases the DMA queue
            sc_ps0 = accp.tile([NH, S], F32, tag="acc", name="sc_ps0")
            scores_half(0, sc_ps0, 0)
            scores_half(0, sc_ps0, 1)
            exp_half(0, sc_ps0, 0)
            exp_half(0, sc_ps0, 1)
            pt_block(0)
            recip0 = rho_block(0)
            for _ in range(5):
                nc.tensor.matmul(warm_ps[:, :], ident, kw16[:, :])
            sc_ps1 = accp.tile([NH, S], F32, tag="acc", name="sc_ps1")
            scores_half(1, sc_ps1, 0)
            scores_half(1, sc_ps1, 1)
            exp_half(1, sc_ps1, 0)
            exp_half(1, sc_ps1, 1)
            r_ps0 = accp.tile([NH, HP], F32, tag="acc", name="r_ps0")
            r_chunks(0, r_ps0, 0, 4)
            pt_block(1)
            recip1 = rho_block(1)
            r_chunks(0, r_ps0, 4, 8)
            r_ps1 = accp.tile([NH, HP], F32, tag="acc", name="r_ps1")
            r_chunks(1, r_ps1, 0, 4)
            r_chunks(1, r_ps1, 4, 6)
            r_chunks(1, r_ps1, 6, 8)
            finish(0, r_ps0, recip0)
            finish(1, r_ps1, recip1)

    nc.finalize()
    return nc


_NC_CACHE = None


def _get_program():
    global _NC_CACHE
    if _NC_CACHE is None:
        _NC_CACHE = build_program()
    return _NC_CACHE


def _host_prep(inputs):
    """Weight fusion + fp16 cast + layout prep (host side)."""
    hs = np.asarray(inputs["hidden_states"], np.float32)
    am = np.asarray(inputs["attention_mask"], np.float32)
    Wq = np.asarray(inputs["Wq"], np.float32)
    bq = np.asarray(inputs["bq"], np.float32)
    Wk = np.asarray(inputs["Wk"], np.float32)
    Wv = np.asarray(inputs["Wv"], np.float32)
    bv = np.asarray(inputs["bv"], np.float32)
    Wo = np.asarray(inputs["Wo"], np.float32)
    bo = np.asarray(inputs["bo"], np.float32)

    wq16 = np.ascontiguousarray(
        Wq.astype(NP16).reshape(C6, 128, H).transpose(1, 0, 2)
    )
    wkt16 = np.ascontiguousarray(
        Wk.T.astype(NP16).reshape(C6, 128, H).transpose(1, 0, 2)
    )

    # g48[p, o, c*8+h] = G_h[128c+p, o],  G_h = Wv[:, h] @ Wo[h, :]
    g48 = np.zeros((128, O, C6 * NH), NP16)
    for h in range(NH):
        Gh = (Wv[:, DH * h : DH * (h + 1)] @ Wo[DH * h : DH * (h + 1), :]).astype(
            NP16
        )
        for c in range(C6):
            g48[:, :, c * NH + h] = Gh[128 * c : 128 * (c + 1), :]

    boeff = (bo + bv @ Wo).astype(np.float32)

    # qmask[p, c*8+h]: 1/sqrt(96) where hidden index 128c+p is in head h
    j = np.arange(H)
    qm = np.zeros((H, NH), np.float32)
    qm[j, j // DH] = 1.0 / np.sqrt(np.float32(DH))
    qm = qm.reshape(C6, 128, NH).transpose(1, 0, 2).reshape(128, C6 * NH)

    kw32 = np.zeros((128, KV_LEN), np.float32)
    kw32[:, KV_QMASK:KV_ONES] = qm
    kw32[:, KV_ONES] = 1.0
    kw32[0, KV_BOEFF : KV_BOEFF + O] = boeff
    kw32[:, KV_NEG4] = -4.0
    kw32[:, KV_ID32 : KV_ID32 + 128] = np.eye(128, dtype=np.float32)

    kw16_base = np.zeros((128, KW_LEN), NP16)
    kw16_base[:, KW_IDENT : KW_IDENT + 128] = np.eye(128, dtype=NP16)
    kw16_base[:, KW_G:KW_ONE] = g48.reshape(128, O * C6 * NH)
    kw16_base[:, KW_ONE] = 1.0

    bq2 = np.broadcast_to(bq, (BL, H)).astype(np.float32).copy()

    hs16 = hs.astype(NP16)

    in_maps = []
    for core in range(NCORES):
        b0 = BL * core
        xpad = np.zeros((BL, S, HP), NP16)
        xpad[:, :, :H] = hs16[b0 : b0 + BL]
        xpad[:, :, RCOL] = 1.0
        xpad = np.ascontiguousarray(
            xpad.reshape(BL, K8, 128, HP).transpose(0, 2, 1, 3)
        )
        xt = np.ascontiguousarray(
            hs16[b0 : b0 + BL]
            .transpose(0, 2, 1)
            .reshape(BL, C6, 128, S)
            .transpose(0, 2, 1, 3)
        )

        kw16 = kw16_base.copy()
        # x0t[p, c*BL+b] = X[b0+b, 0, 128c+p]
        kw16[:, KW_X0T:KW_G] = (
            hs16[b0 : b0 + BL, 0, :]
            .reshape(BL, C6, 128)
            .transpose(2, 1, 0)
            .reshape(128, C6 * BL)
        )

        # am[p, b*K8+k] = mask[b0+b, 128k+p]
        amc = (
            am[b0 : b0 + BL, :]
            .reshape(BL, K8, 128)
            .transpose(2, 0, 1)
            .reshape(128, BL * K8)
            .astype(NP16)
        )

        in_maps.append(
            {
                "x": xpad,
                "xt": xt,
                "wq": wq16,
                "wkt": wkt16,
                "kw16": kw16,
                "kw32": kw32,
                "am": np.ascontiguousarray(amc),
                "bq2": bq2,
            }
        )
    return in_maps


def kernel(**inputs) -> np.ndarray:
    nc = _get_program()
    in_maps = _host_prep(inputs)
    res = run_bass_kernel_spmd(nc, in_maps, core_ids=list(range(NCORES)))
    return np.concatenate([r["out"] for r in res.results], axis=0).astype(np.float32)


if __name__ == "__main__":
    rng = np.random.default_rng(0)
    demo = {
        "hidden_states": rng.standard_normal((B, S, H), dtype=np.float32),
        "attention_mask": np.ones((B, S), np.float32),
        "Wq": rng.standard_normal((H, H), dtype=np.float32) / np.sqrt(H),
        "bq": np.zeros(H, np.float32),
        "Wk": rng.standard_normal((H, H), dtype=np.float32) / np.sqrt(H),
        "bk": np.zeros(H, np.float32),
        "Wv": rng.standard_normal((H, H), dtype=np.float32) / np.sqrt(H),
        "bv": np.zeros(H, np.float32),
        "Wo": rng.standard_normal((H, O), dtype=np.float32) / np.sqrt(H),
        "bo": np.zeros(O, np.float32),
    }
    out = kernel(**demo)
    print(out.shape, out.dtype)


# revision 21
# speedup vs baseline: 1.0271x; 1.0271x over previous
"""Trainium2 Bass kernel for BERT-style CLS attention head (v2: fp16 dual-layout).

Model (see harness reference):
  q/k/v projections of hidden [B=16, S=1024, H=768], 8 heads x 96,
  softmax attention, but ONLY the CLS token (query position 0) feeds the
  output projection  out = relu(ctx[:, 0] @ Wo + bo)  with Wo [768, 4].

Algebraic structure exploited on-device (per batch b):
  q~      = X[0] @ Wq + bq                  (only row 0 of Q needed)
  Qblk    [768, 16] = diag-blocked q~/sqrt(96)
  Z^T     [16, 768] = Qblk.T @ Wk^T         (K-projection collapses to a
                                             rank-16 op; bk cancels in
                                             softmax)
  scores  [8, 1024] = Z.T @ X^T             (X^T comes pre-transposed
                                             from DRAM - no on-chip
                                             transposes of X at all)
  probs   = exp(scores - 4)                 (unnormalized; shift cancels)
  pt      = probs^T * mask                  (mask folded into the PSUM
                                             evacuation multiply)
  r       [8, 769]  = pt.T @ [X | 1]       (ones-column of X makes
                                             r[:,768] = rowsum -> softmax
                                             denominator for free)
  out     [4]       = relu(sum_hc rt*g/rho + boeff)  (DVE mult-reduce +
                                             one fp32 matmul; G_h =
                                             Wv_h @ Wo_h host-fused)

All streamed tensors are fp16 (host-side cast + layout only; fp32
accumulation in PSUM). Per-core HBM traffic ~8.6 MB -> ~24 us DMA bound.
Sharding: data-parallel over batch, 2 batches per core on 8 cores.
"""

import numpy as np

from concourse import bacc
import concourse.mybir as mybir
import concourse.tile as tile
from concourse.bass import _add_dep_helper
from concourse.bass_utils import run_bass_kernel_spmd

F32 = mybir.dt.float32
import ml_dtypes
NP16 = ml_dtypes.bfloat16
F16 = mybir.dt.bfloat16
F32R = mybir.dt.float32r


def _r(ap):
    return ap.bitcast(F32R)

B, S, H = 16, 1024, 768
NH, DH, O = 8, 96, 4
NCORES = 8
BL = B // NCORES          # 2 batches per core
C6 = H // 128             # 6 hidden chunks of 128
K8 = S // 128             # 8 sequence chunks of 128
HP = 772                  # padded hidden: col 768 = 1.0 (rowsum), 769.. = 0
RCOL = H                  # index of the ones column in padded X

# kw16 packing [128, .] fp16: ident | x0t | g48
KW_IDENT = 0
KW_X0T = 128                       # x0t[p, c*BL + b]
KW_G = KW_X0T + C6 * BL            # 140; g48[p, o*48 + c*NH + h]
KW_ONE = KW_G + O * C6 * NH        # 332: fp16 ones column
KW_BOE = KW_ONE + 1                # 333: boeff (partition 0)
KW_LEN = KW_BOE + O + 3            # 340

# kw32 packing [128, .] fp32: qmask | ones col | boeff (partition 0)
KV_QMASK = 0                       # qmask[p, c*NH + h]
KV_ONES = C6 * NH                  # 48
KV_BOEFF = KV_ONES + 1             # 49 (partition 0 only)
KV_NEG4 = KV_BOEFF + O             # 53: exp bias (-4.0, all partitions)
KV_ID32 = KV_NEG4 + 1              # 54
KV_LEN = KV_ID32 + 128             # 182


def build_program():
    nc = bacc.Bacc(None)

    x_d = nc.declare_dram_parameter("x", [BL, 128, K8, HP], F16, isOutput=False)
    xt_d = nc.declare_dram_parameter("xt", [BL, 128, C6, S], F16, isOutput=False)
    wq_d = nc.declare_dram_parameter("wq", [128, C6, H], F16, isOutput=False)
    wkt_d = nc.declare_dram_parameter("wkt", [128, C6, H], F16, isOutput=False)
    kw16_d = nc.declare_dram_parameter("kw16", [128, KW_LEN], F16, isOutput=False)
    kw32_d = nc.declare_dram_parameter("kw32", [128, KV_LEN], F32, isOutput=False)
    am_d = nc.declare_dram_parameter("am", [128, BL * K8], F16, isOutput=False)
    bq_d = nc.declare_dram_parameter("bq2", [BL, H], F32, isOutput=False)
    out_d = nc.declare_dram_parameter("out", [BL, O], F32, isOutput=True)

    with tile.TileContext(nc) as tc:
        with (
            tc.tile_pool(name="konst", bufs=1) as kp,
            tc.tile_pool(name="work", bufs=1) as wp,
            tc.tile_pool(name="tps", bufs=2, space="PSUM") as tpsp,
            tc.tile_pool(name="acc", bufs=2, space="PSUM") as accp,
            tc.tile_pool(name="jnk", bufs=1, space="PSUM") as jp,
            tc.tile_pool(name="oup", bufs=1, space="PSUM") as op_,
        ):
            # ---- persistent SBUF tiles ----
            kw16 = kp.tile([128, KW_LEN], F16)
            kw32 = kp.tile([128, KV_LEN], F32)
            am16 = kp.tile([128, BL * K8], F16)
            bq32 = kp.tile([BL, H], F32)
            wq_sb = kp.tile([128, C6, H], F16)
            wkt_sb = kp.tile([128, C6, H], F16)
            x_sb = kp.tile([128, BL, K8, HP], F16)
            xt_sb = kp.tile([128, BL, C6, S], F16)

            ident = kw16[:, KW_IDENT : KW_IDENT + 128]
            x0t_v = kw16[:, KW_X0T : KW_G].rearrange("p (c b) -> p c b", c=C6)
            g48_v = kw16[:, KW_G : KW_ONE].rearrange("p (o a) -> p o a", o=O)
            qmask_v = kw32[:, KV_QMASK : KV_ONES].rearrange("p (c h) -> p c h", c=C6)
            ones_v = kw32[:, KV_ONES : KV_ONES + 1]
            one16_v = kw16[:, KW_ONE : KW_ONE + 1]
            boeff16_v = kw16[0:1, KW_BOE : KW_BOE + O]
            boeff_v = kw32[0:1, KV_BOEFF : KV_BOEFF + O]
            id32 = kw32[:, KV_ID32 : KV_ID32 + 128]
            neg4_v = kw32[:, KV_NEG4 : KV_NEG4 + 1]

            # ---- DMA queue (HWDGE; priority-chained, 2 in flight) ----
            dmas = []
            dmas.append(nc.sync.dma_start(out=kw16[:, :], in_=kw16_d[:, :]))
            dmas.append(nc.sync.dma_start(out=kw32[:, :], in_=kw32_d[:, :]))
            dmas.append(nc.sync.dma_start(out=am16[:, :], in_=am_d[:, :]))
            dmas.append(nc.sync.dma_start(out=bq32[:, :], in_=bq_d[:, :]))
            d_wq = []
            d_wkt = []
            for half, (c0, cn) in enumerate(((0, 3), (3, 6))):
                d_wq.append(
                    nc.sync.dma_start(
                        out=wq_sb[:, c0:cn, :],
                        in_=wq_d[:, c0:cn, :],
                    )
                )
            for half, (c0, cn) in enumerate(((0, 3), (3, 6))):
                d_wkt.append(
                    nc.sync.dma_start(
                        out=wkt_sb[:, c0:cn, :],
                        in_=wkt_d[:, c0:cn, :],
                    )
                )
            dmas.extend(d_wq)
            dmas.extend(d_wkt)

            d_xt = {}   # (b, half) -> dma
            d_x = {}    # (b, piece) -> dma; pieces in s-chunks of 128

            def load_xt(b, nh2):
                d_xt[(b, nh2)] = nc.sync.dma_start(
                    out=xt_sb[:, b, :, 512 * nh2 : 512 * (nh2 + 1)],
                    in_=xt_d[b].rearrange("(c p) s -> p c s", p=128)[
                        :, :, 512 * nh2 : 512 * (nh2 + 1)
                    ],
                )

            def load_x(b, k0, kn):
                d_x[(b, k0)] = nc.sync.dma_start(
                    out=x_sb[:, b, k0:kn, :],
                    in_=x_d[b, :, k0:kn, :],
                )

            def load_xt_full(b):
                d_xt[b] = nc.sync.dma_start(
                    out=xt_sb[:, b, :, :],
                    in_=xt_d[b, :, :, :],
                )

            load_xt_full(0)
            load_xt_full(1)
            load_x(0, 0, 4)
            load_x(0, 4, 8)
            load_x(1, 0, 4)
            load_x(1, 4, 6)
            load_x(1, 6, 8)
            dmas.extend(
                [
                    d_xt[0], d_xt[1], d_x[(0, 0)], d_x[(0, 4)],
                    d_x[(1, 0)], d_x[(1, 4)], d_x[(1, 6)],
                ]
            )
            # keep four transfers in flight: hides the ~2.2us per-transfer
            # HWDGE gen + completion-receipt latency while the single ring's
            # FIFO drain keeps bytes landing in priority order
            for i in range(4, len(dmas)):
                _add_dep_helper(
                    dmas[i].ins, dmas[i - 4].ins, sync=True, reason="dma order"
                )

            # ---- PE warmup (HAM unthrottle) while weights stream ----
            warm_ps = jp.tile([128, KW_LEN], F32)
            for _ in range(12):
                nc.tensor.matmul(warm_ps[:, :], ident, kw16[:, :])

            # ---- q~ = X[:,0,:] @ Wq + bq : [BL, H] ----
            q_ps = accp.tile([BL, H], F32, tag="acc", name="q_ps")
            for c in range(C6):
                for n0, nw in ((0, 512), (512, 256)):
                    nc.tensor.matmul(
                        q_ps[:, n0 : n0 + nw],
                        x0t_v[:, c, :],
                        wq_sb[:, c, n0 : n0 + nw],
                        start=(c == 0),
                        stop=(c == C6 - 1),
                    )
            q_sb = wp.tile([BL, H], F32)
            nc.vector.tensor_add(q_sb[:, :], q_ps[:, :], bq32[:, :])

            # ---- qT via PE transposes, fused into Qblk = qT * headmask ----
            qt_ps = tpsp.tile([128, 512], F32, tag="tps", name="qt_ps")
            for c in range(C6):
                nc.tensor.transpose(
                    qt_ps[:, BL * c : BL * (c + 1)],
                    q_sb[:, 128 * c : 128 * (c + 1)],
                    id32[:BL, :BL],
                )
            qblk = wp.tile([128, C6, BL, NH], F16)
            nc.vector.tensor_mul(
                qblk[:, :, :, :],
                qt_ps[:, : C6 * BL]
                .rearrange("p (c b) -> p c b", c=C6)
                .unsqueeze(3)
                .to_broadcast([128, C6, BL, NH]),
                qmask_v.unsqueeze(2).to_broadcast([128, C6, BL, NH]),
            )

            # ---- Z^T [16, 768] = Qblk.T @ WkT, then Z [768, 16] ----
            zt_ps = accp.tile([BL * NH, H], F32, tag="acc", name="zt_ps")
            for jc in range(C6):
                for n0, nw in ((0, 512), (512, 256)):
                    nc.tensor.matmul(
                        zt_ps[:, n0 : n0 + nw],
                        qblk[:, jc, :, :].rearrange("p b h -> p (b h)"),
                        wkt_sb[:, jc, n0 : n0 + nw],
                        start=(jc == 0),
                        stop=(jc == C6 - 1),
                    )
            zt_sb = wp.tile([BL * NH, H], F32)
            nc.vector.tensor_copy(zt_sb[:, :], zt_ps[:, :])
            z_tps = tpsp.tile([128, 512], F32, tag="tps", name="z_tps")
            for c in range(C6):
                nc.tensor.transpose(
                    z_tps[:, 16 * c : 16 * (c + 1)],
                    zt_sb[:, 128 * c : 128 * (c + 1)],
                    id32[: BL * NH, : BL * NH],
                )
            z_sb = wp.tile([128, C6, BL * NH], F16)
            nc.vector.tensor_copy(z_sb[:, :, :], z_tps[:, : C6 * BL * NH].rearrange("p (c a) -> p c a", c=C6))

            # ---- per-batch attention pipeline ----
            probs = wp.tile([NH, BL, S], F32)
            pt_sb = wp.tile([128, BL, K8, NH], F16)

            def scores_half(b, sc_ps, nh2):
                for ic in range(C6):
                    nc.tensor.matmul(
                        sc_ps[:, 512 * nh2 : 512 * (nh2 + 1)],
                        z_sb[:, ic, NH * b : NH * (b + 1)],
                        xt_sb[:, b, ic, 512 * nh2 : 512 * (nh2 + 1)],
                        start=(ic == 0),
                        stop=(ic == C6 - 1),
                    )

            def exp_half(b, sc_ps, nh2):
                # shift-invariant exp; -4 guards fp16 range (|score| <~ 7)
                nc.scalar.activation(
                    probs[:, b, 512 * nh2 : 512 * (nh2 + 1)],
                    sc_ps[:, 512 * nh2 : 512 * (nh2 + 1)],
                    mybir.ActivationFunctionType.Exp,
                    bias=neg4_v[:NH, :],
                    scale=1.0,
                )

            def pt_block(b):
                pt_ps = tpsp.tile([128, 512], F32, tag="tps", name=f"pt_ps{b}")
                for k in range(K8):
                    nc.tensor.transpose(
                        pt_ps[:, NH * k : NH * (k + 1)],
                        probs[:, b, 128 * k : 128 * (k + 1)],
                        id32[:NH, :NH],
                    )
                # attention mask folded into the PSUM evacuation (exact:
                # exp(score - 10000) == 0 == exp(score) * mask in fp32)
                nc.vector.tensor_mul(
                    pt_sb[:, b, :, :],
                    pt_ps[:, : K8 * NH].rearrange("p (k h) -> p k h", k=K8),
                    am16[:, b * K8 : (b + 1) * K8]
                    .unsqueeze(2)
                    .to_broadcast([128, K8, NH]),
                )

            def rho_block(b):
                rho_ps = op_.tile([1, NH], F32, tag="out", name=f"rho{b}")
                for k in range(K8):
                    nc.tensor.matmul(
                        rho_ps[:, :],
                        one16_v,
                        pt_sb[:, b, k, :],
                        start=(k == 0),
                        stop=(k == K8 - 1),
                    )
                rho_sb = wp.tile([1, NH], F32, name=f"rho_sb{b}")
                nc.vector.tensor_copy(rho_sb[:, :], rho_ps[:, :])
                rt_ps_t = tpsp.tile([128, 512], F32, tag="tps", name=f"rhot{b}")
                nc.tensor.transpose(
                    rt_ps_t[:NH, :1], rho_sb[:, :], id32[:1, :1]
                )
                recip = wp.tile([NH, 1], F32, name=f"recip{b}")
                nc.vector.reciprocal(recip[:, :], rt_ps_t[:NH, :1])
                return recip

            def r_chunks(b, r_ps, k0, kn):
                for k in range(k0, kn):
                    for n0, nw in ((0, 512), (512, HP - 512)):
                        nc.tensor.matmul(
                            r_ps[:, n0 : n0 + nw],
                            pt_sb[:, b, k, :],
                            x_sb[:, b, k, n0 : n0 + nw],
                            start=(k == 0),
                            stop=(k == K8 - 1),
                        )

            def finish(b, r_ps, recip):
                rsc = wp.tile([NH, H], F32, name=f"rsc{b}")
                nc.vector.tensor_scalar_mul(
                    rsc[:, 0:384], r_ps[:, 0:384], recip[:, :]
                )
                nc.scalar.activation(
                    rsc[:, 384:H],
                    r_ps[:, 384:H],
                    mybir.ActivationFunctionType.Copy,
                    bias=0.0,
                    scale=recip[:, :],
                )
                rt_ps = tpsp.tile([128, 512], F32, tag="tps", name=f"rt_ps{b}")
                for c in range(C6):
                    nc.tensor.transpose(
                        rt_ps[:, NH * c : NH * (c + 1)],
                        rsc[:, 128 * c : 128 * (c + 1)],
                        id32[:NH, :NH],
                    )
                rt_sb = wp.tile([128, C6 * NH], F32, name=f"rt_sb{b}")
                nc.vector.tensor_copy(rt_sb[:, :], rt_ps[:, : C6 * NH])
                scrap = wp.tile([128, O, C6 * NH], F32, name=f"scrap{b}")
                partials = wp.tile([128, O], F32, name=f"partials{b}")
                nc.vector.tensor_mul(
                    scrap[:, :, :],
                    rt_sb[:, :].unsqueeze(1).to_broadcast([128, O, C6 * NH]),
                    g48_v[:, :, :],
                )
                nc.vector.tensor_reduce(
                    partials[:, :].unsqueeze(2),
                    scrap[:, :, :],
                    mybir.AxisListType.X,
                    mybir.AluOpType.add,
                )
                osum_ps = op_.tile([1, O], F32, tag="out", name=f"osum{b}")
                nc.tensor.matmul(
                    osum_ps[:, :],
                    one16_v[0:1, :],
                    boeff16_v,
                    start=True,
                    stop=False,
                )
                nc.tensor.matmul(
                    osum_ps[:, :], ones_v, partials[:, :], start=False, stop=True
                )
                out_sb = wp.tile([1, O], F32, name=f"out_sb{b}")
                nc.vector.tensor_scalar_max(out_sb[:, :], osum_ps[:, :], 0.0)
                nc.scalar.dma_start(out=out_d[b : b + 1, :], in_=out_sb[:, :])

            # interleaved two-batch pipeline: PE chases the DMA queue
            sc_ps0 = accp.tile([NH, S], F32, tag="acc", name="sc_ps0")
            scores_half(0, sc_ps0, 0)
            scores_half(0, sc_ps0, 1)
            exp_half(0, sc_ps0, 0)
            exp_half(0, sc_ps0, 1)
            pt_block(0)
            recip0 = rho_block(0)
            for _ in range(5):
                nc.tensor.matmul(warm_ps[:, :], ident, kw16[:, :])
            sc_ps1 = accp.tile([NH, S], F32, tag="acc", name="sc_ps1")
            scores_half(1, sc_ps1, 0)
            scores_half(1, sc_ps1, 1)
            exp_half(1, sc_ps1, 0)
            exp_half(1, sc_ps1, 1)
            r_ps0 = accp.tile([NH, HP], F32, tag="acc", name="r_ps0")
            r_chunks(0, r_ps0, 0, 4)
            pt_block(1)
            recip1 = rho_block(1)
            r_chunks(0, r_ps0, 4, 8)
            r_ps1 = accp.tile([NH, HP], F32, tag="acc", name="r_ps1")
            r_chunks(1, r_ps1, 0, 4)
            r_chunks(1, r_ps1, 4, 6)
            r_chunks(1, r_ps1, 6, 8)
            finish(0, r_ps0, recip0)
            finish(1, r_ps1, recip1)

    nc.finalize()
    return nc


_NC_CACHE = None


def _get_program():
    global _NC_CACHE
    if _NC_CACHE is None:
        _NC_CACHE = build_program()
    return _NC_CACHE


def _host_prep(inputs):
    """Weight fusion + fp16 cast + layout prep (host side)."""
    hs = np.asarray(inputs["hidden_states"], np.float32)
    am = np.asarray(inputs["attention_mask"], np.float32)
    Wq = np.asarray(inputs["Wq"], np.float32)
    bq = np.asarray(inputs["bq"], np.float32)
    Wk = np.asarray(inputs["Wk"], np.float32)
    Wv = np.asarray(inputs["Wv"], np.float32)
    bv = np.asarray(inputs["bv"], np.float32)
    Wo = np.asarray(inputs["Wo"], np.float32)
    bo = np.asarray(inputs["bo"], np.float32)

    wq16 = np.ascontiguousarray(
        Wq.astype(NP16).reshape(C6, 128, H).transpose(1, 0, 2)
    )
    wkt16 = np.ascontiguousarray(
        Wk.T.astype(NP16).reshape(C6, 128, H).transpose(1, 0, 2)
    )

    # g48[p, o, c*8+h] = G_h[128c+p, o],  G_h = Wv[:, h] @ Wo[h, :]
    g48 = np.zeros((128, O, C6 * NH), NP16)
    for h in range(NH):
        Gh = (Wv[:, DH * h : DH * (h + 1)] @ Wo[DH * h : DH * (h + 1), :]).astype(
            NP16
        )
        for c in range(C6):
            g48[:, :, c * NH + h] = Gh[128 * c : 128 * (c + 1), :]

    boeff = (bo + bv @ Wo).astype(np.float32)

    # qmask[p, c*8+h]: 1/sqrt(96) where hidden index 128c+p is in head h
    j = np.arange(H)
    qm = np.zeros((H, NH), np.float32)
    qm[j, j // DH] = 1.0 / np.sqrt(np.float32(DH))
    qm = qm.reshape(C6, 128, NH).transpose(1, 0, 2).reshape(128, C6 * NH)

    kw32 = np.zeros((128, KV_LEN), np.float32)
    kw32[:, KV_QMASK:KV_ONES] = qm
    kw32[:, KV_ONES] = 1.0
    kw32[0, KV_BOEFF : KV_BOEFF + O] = boeff
    kw32[:, KV_NEG4] = -4.0
    kw32[:, KV_ID32 : KV_ID32 + 128] = np.eye(128, dtype=np.float32)

    kw16_base = np.zeros((128, KW_LEN), NP16)
    kw16_base[:, KW_IDENT : KW_IDENT + 128] = np.eye(128, dtype=NP16)
    kw16_base[:, KW_G:KW_ONE] = g48.reshape(128, O * C6 * NH)
    kw16_base[:, KW_ONE] = 1.0
    kw16_base[0, KW_BOE : KW_BOE + O] = boeff.astype(NP16)

    bq2 = np.broadcast_to(bq, (BL, H)).astype(np.float32).copy()

    hs16 = hs.astype(NP16)

    in_maps = []
    for core in range(NCORES):
        b0 = BL * core
        xpad = np.zeros((BL, S, HP), NP16)
        xpad[:, :, :H] = hs16[b0 : b0 + BL]
        xpad[:, :, RCOL] = 1.0
        xpad = np.ascontiguousarray(
            xpad.reshape(BL, K8, 128, HP).transpose(0, 2, 1, 3)
        )
        xt = np.ascontiguousarray(
            hs16[b0 : b0 + BL]
            .transpose(0, 2, 1)
            .reshape(BL, C6, 128, S)
            .transpose(0, 2, 1, 3)
        )

        kw16 = kw16_base.copy()
        # x0t[p, c*BL+b] = X[b0+b, 0, 128c+p]
        kw16[:, KW_X0T:KW_G] = (
            hs16[b0 : b0 + BL, 0, :]
            .reshape(BL, C6, 128)
            .transpose(2, 1, 0)
            .reshape(128, C6 * BL)
        )

        # am[p, b*K8+k] = mask[b0+b, 128k+p]
        amc = (
            am[b0 : b0 + BL, :]
            .reshape(BL, K8, 128)
            .transpose(2, 0, 1)
            .reshape(128, BL * K8)
            .astype(NP16)
        )

        in_maps.append(
            {
                "x": xpad,
                "xt": xt,
                "wq": wq16,
                "wkt": wkt16,
                "kw16": kw16,
                "kw32": kw32,
                "am": np.ascontiguousarray(amc),
                "bq2": bq2,
            }
        )
    return in_maps


def kernel(**inputs) -> np.ndarray:
    nc = _get_program()
    in_maps = _host_prep(inputs)
    res = run_bass_kernel_spmd(nc, in_maps, core_ids=list(range(NCORES)))
    return np.concatenate([r["out"] for r in res.results], axis=0).astype(np.float32)


if __name__ == "__main__":
    rng = np.random.default_rng(0)
    demo = {
        "hidden_states": rng.standard_normal((B, S, H), dtype=np.float32),
        "attention_mask": np.ones((B, S), np.float32),
        "Wq": rng.standard_normal((H, H), dtype=np.float32) / np.sqrt(H),
        "bq": np.zeros(H, np.float32),
        "Wk": rng.standard_normal((H, H), dtype=np.float32) / np.sqrt(H),
        "bk": np.zeros(H, np.float32),
        "Wv": rng.standard_normal((H, H), dtype=np.float32) / np.sqrt(H),
        "bv": np.zeros(H, np.float32),
        "Wo": rng.standard_normal((H, O), dtype=np.float32) / np.sqrt(H),
        "bo": np.zeros(O, np.float32),
    }
    out = kernel(**demo)
    print(out.shape, out.dtype)
